# revision 18
# baseline (speedup 1.0000x reference)
"""Trainium2 Bass kernel for a basic RNN LM step:
    xe = emb[x]; xw = xe @ w_ih + b_h
    h_t = tanh(xw_t + h_{t-1} @ w_hh)   (sequential over S=256)
    out = hs @ w_ho + b_o               (dominant GEMM, vocab-sharded)

Split design over 8 NeuronCores (single SPMD program, branches on core id):
  - Phase A (uniform SPMD): each core embeds + transposes + input-projects its
    own 512-token slice (per-core x_idx input), then an AllGather shares the
    xw^T blocks.  Core 0's own block is local, so the recurrence starts
    immediately.
  - Core 0 runs the sequential recurrence (a latency chain of 64 fp16
    weight-reload matmuls per step).  Every 32 steps it broadcasts the new
    hs^T block (1 MB fp16) via AllReduce (other cores contribute zeros).
  - Cores 1..7 tensor-parallel the output projection over vocab columns
    (4572 each, padded to 4608), consuming hs^T blocks as they arrive; w_ho
    is pre-cast to fp16 in DRAM during their initial idle window.
  - All matmuls run in fp16 (1 cycle/row + fast weight load) with fp32 PSUM
    accumulation; measured logit error vs the fp32 reference is ~5e-4
    (absmax-relative).
  - The hidden state is kept transposed (h^T tiles [128, kc, 16]) so the
    recurrence needs no per-step transposes; hs^T blocks double as the
    stationary operand of the projection.
"""

import numpy as np

import concourse.bass as bass
import concourse.mybir as mybir
import concourse.tile as tile
from concourse import bacc
from concourse.bass import ts
from concourse.bass_utils import run_bass_kernel_spmd
from concourse.masks import make_identity

F32 = mybir.dt.float32
F16 = mybir.dt.float16
I32 = mybir.dt.int32

VOCAB = 32000
H = 1024
B = 16
S = 256
NCORES = 8
NPROJ = NCORES - 1  # 7 projection cores
VSH = 4608  # padded per-core vocab shard (ceil(32000/7)=4572 -> 9*512)
VREAL = 4572  # real columns per projection core
KC = H // 128  # 8 chunks of the hidden dim
BS = B * S  # 4096 (time-major: index = s*B + b)
NT = VSH // 512  # 9 projection column tiles of 512
SB = 32  # recurrence steps per broadcast batch / phase-A block
NBATCH = S // SB  # 8 batches == 8 blocks, one per core
Tanh = mybir.ActivationFunctionType.Tanh

_cache = {}


def _install_profile_hook():
    """Make trace=True work under axon if the antenv hook module is absent."""
    import sys
    import types

    try:
        import antenv.axon_hooks  # noqa: F401

        return
    except ImportError:
        pass
    mod = types.ModuleType("antenv.axon_hooks")
    mod._hook = None

    def set_axon_ntff_profile_hook(hook):
        mod._hook = hook

    def get_axon_ntff_profile_hook():
        return mod._hook

    mod.set_axon_ntff_profile_hook = set_axon_ntff_profile_hook
    mod.get_axon_ntff_profile_hook = get_axon_ntff_profile_hook
    sys.modules["antenv.axon_hooks"] = mod
    try:
        from trn_agent_boot.trn_boot import _ntff_profile_via_ctypes

        hook = _ntff_profile_via_ctypes("/opt/axon/libaxon_pjrt.so")
        if hook is not None:
            mod._hook = hook
    except Exception:
        pass


def _build():
    nc = bacc.Bacc("TRN2", target_bir_lowering=False, debug=False, num_devices=NCORES)

    # ---- I/O ----
    # per-core slice of token ids: 512 tokens [128, 4] (core c: tokens c*512..)
    x_idx = nc.declare_dram_parameter("x_idx", [128, 4], I32, isOutput=False)
    hT0_in = nc.declare_dram_parameter("hT0", [H, B], F32, isOutput=False)
    emb_in = nc.declare_dram_parameter("emb", [VOCAB, H], F32, isOutput=False)
    wih_in = nc.declare_dram_parameter("w_ih", [H, H], F32, isOutput=False)
    whh_in = nc.declare_dram_parameter("w_hh", [H, H], F32, isOutput=False)
    bh_in = nc.declare_dram_parameter("b_h", [H], F32, isOutput=False)
    who_in = nc.declare_dram_parameter("w_ho", [H, VSH], F32, isOutput=False)
    bo_in = nc.declare_dram_parameter("b_o", [VSH], F32, isOutput=False)
    out_d = nc.declare_dram_parameter("out", [BS, VSH], F32, isOutput=True)
    hT_out = nc.declare_dram_parameter("hT_fin", [128, KC, B], F32, isOutput=True)

    # internal DRAM
    xwblk_d = [
        nc.dram_tensor(f"xwblk{i}", [128, KC, w], F16)
        for i, w in enumerate((128, 128, 256))
    ]  # local xw^T block, split to keep collective inputs contiguous
    # xw^T AllGathers split by column range so early steps arrive first
    xw_ag_d = [
        nc.dram_tensor(f"xw_ag{i}", [NCORES, 128, KC, w], F16, addr_space="Shared")
        for i, w in enumerate((128, 128, 256))
    ]
    who16_d = nc.dram_tensor("who16", [H, VSH], F16)  # fp16 copy of w_ho shard
    who16_r = who16_d.ap().rearrange("(c p) v -> p c v", p=128)
    bin_d = [nc.dram_tensor(f"hsb_in{i}", [128, KC, SB * B], F16) for i in range(NBATCH)]
    bout_d = [
        nc.dram_tensor(f"hsb_out{i}", [128, KC, SB * B], F16, addr_space="Shared")
        for i in range(NBATCH)
    ]

    who_r = who_in.rearrange("(c p) v -> p c v", p=128)
    emb_ap = emb_in[:]
    warm_in_d = nc.dram_tensor("ccw_in", [1, 64], F32)
    warm_out_d = nc.dram_tensor("ccw_out", [1, 64], F32, addr_space="Shared")

    with tile.TileContext(nc) as tc:
        pid = nc.partition_id()
        with (
            tc.tile_pool(name="consts", bufs=1) as consts,
            tc.tile_pool(name="wstage", bufs=3) as wstage,
            tc.tile_pool(name="gather", bufs=2) as gather_p,
            tc.tile_pool(name="xe16", bufs=2) as xe16_p,
            tc.tile_pool(name="xeT", bufs=1) as xeT_p,
            tc.tile_pool(name="xwt", bufs=6) as xwt_p,
            tc.tile_pool(name="hswin", bufs=2) as hswin_p,
            tc.tile_pool(name="zpre", bufs=8) as zpre_p,
            tc.tile_pool(name="who", bufs=2) as who_p,
            tc.tile_pool(name="hsrx", bufs=2) as hsrx_p,
            tc.tile_pool(name="evac", bufs=2) as evac_p,
            tc.tile_pool(name="ps_t", bufs=1, space="PSUM") as ps_t,
            tc.tile_pool(name="ps_xw", bufs=2, space="PSUM") as ps_xw,
            tc.tile_pool(name="ps_rec", bufs=3, space="PSUM") as ps_rec,
            tc.tile_pool(name="ps_prj", bufs=2, space="PSUM") as ps_prj,
        ):
            # warm up the collective core early so the xw AllGathers are not
            # delayed by its ~70us spin-up
            warm_sb = wstage.tile([1, 64], F32, tag="ccw")
            nc.vector.memset(warm_sb[:], 0.0)
            nc.sync.dma_start(warm_in_d[:], warm_sb[:])
            nc.gpsimd.collective_compute(
                "AllReduce",
                mybir.AluOpType.add,
                replica_groups=[list(range(NCORES))],
                ins=[warm_in_d[:]],
                outs=[warm_out_d[:]],
            )

            # ---- constants (all cores) ----
            ident = consts.tile([128, 128], F16, tag="ident")
            make_identity(nc, ident[:])

            idx_sb = consts.tile([128, 4], I32, tag="idx")
            nc.sync.dma_start(idx_sb[:], x_idx[:])

            bh_sb = consts.tile([128, KC], F32, tag="bh")
            nc.sync.dma_start(bh_sb[:], bh_in.rearrange("(c p) -> p c", p=128))

            # w_ih, w_hh -> fp16 SBUF, chunked [128, KC, H]
            wih16 = consts.tile([128, KC, H], F16, tag="wih16")
            whh16 = consts.tile([128, KC, H], F16, tag="whh16")
            for w16, w_in in ((wih16, wih_in), (whh16, whh_in)):
                wr = w_in.rearrange("(c p) m -> p c m", p=128)
                for c in range(KC):
                    st = wstage.tile([128, H], F32, tag="wst")
                    nc.sync.dma_start(st[:], wr[:, c, :])
                    nc.vector.tensor_copy(w16[:, c, :], st[:])

            # initial hidden state h^T -> fp16 [128, KC, B]
            h0st = wstage.tile([128, KC, B], F32, tag="h0st")
            nc.sync.dma_start(h0st[:], hT0_in.rearrange("(c p) b -> p c b", p=128))
            h0_16 = consts.tile([128, KC, B], F16, tag="h0_16")
            nc.vector.tensor_copy(h0_16[:], h0st[:])

            hfin = consts.tile([128, KC, B], F32, tag="hfin")

            # ---- phase A (uniform): embed + transpose + xw GEMM of own block --
            xeT = xeT_p.tile([128, KC, SB * B], F16, tag="xeT")
            for sub in range(4):
                ge = gather_p.tile([128, H], F32, tag="ge")
                nc.gpsimd.indirect_dma_start(
                    out=ge[:],
                    out_offset=None,
                    in_=emb_ap,
                    in_offset=bass.IndirectOffsetOnAxis(
                        ap=idx_sb[:, sub : sub + 1], axis=0
                    ),
                )
                xe16 = xe16_p.tile([128, H], F16, tag="xe16")
                nc.vector.tensor_copy(xe16[:], ge[:])
                for k in range(KC):
                    pt = ps_t.tile([128, 128], F16, tag="pt")
                    nc.tensor.transpose(pt[:], xe16[:, ts(k, 128)], ident[:])
                    nc.vector.tensor_copy(xeT[:, k, ts(sub, 128)], pt[:])
            for m in range(KC):
                pxw = ps_xw.tile([128, SB * B], F32, tag="pxw")
                for k in range(KC):
                    nc.tensor.matmul(
                        pxw[:],
                        lhsT=wih16[:, k, ts(m, 128)],
                        rhs=xeT[:, k, :],
                        start=(k == 0),
                        stop=(k == KC - 1),
                    )
                xv = evac_p.tile([128, SB * B], F16, tag="xv")
                nc.scalar.add(xv[:], pxw[:], bh_sb[:, m : m + 1])
                for agi, (lo, w) in enumerate(((0, 128), (128, 128), (256, 256))):
                    nc.sync.dma_start(xwblk_d[agi][:, m, :], xv[:, lo : lo + w])

            # share the xw blocks: xw_ag*[c] = core c's block (column-split)
            for agi in range(3):
                nc.gpsimd.collective_compute(
                    "AllGather",
                    mybir.AluOpType.bypass,
                    replica_groups=[list(range(NCORES))],
                    ins=[xwblk_d[agi][:]],
                    outs=[xw_ag_d[agi][:]],
                )

            # ---- non-root setup: zero collective inputs, b_o bcast, w_ho cast
            with tc.If(pid != 0) as cmp0:
                zt = consts.tile([128, KC, SB * B], F16, tag="zt")
                nc.vector.memset(zt[:], 0.0)
                for i in range(NBATCH):
                    nc.sync.dma_start(bin_d[i][:], zt[:])
                bo_bc = consts.tile([128, VSH], F32, tag="bo_bc")
                for p in range(128):
                    nc.sync.dma_start(bo_bc[p : p + 1, :], bo_in[:].unsqueeze(0))
                for c in range(KC):
                    for q in range(4):
                        st = wstage.tile([128, VSH // 4], F32, tag="whost")
                        nc.sync.dma_start(st[:], who_r[:, c, ts(q, VSH // 4)])
                        s16 = wstage.tile([128, VSH // 4], F16, tag="whost16")
                        nc.vector.tensor_copy(s16[:], st[:])
                        nc.sync.dma_start(who16_r[:, c, ts(q, VSH // 4)], s16[:])

            # ================= recurrence + broadcast + projection ============
            hw_prev = None
            for bi in range(NBATCH):
                with tc.If(pid == 0) as cmp:
                    hw = hswin_p.tile([128, KC, SB * B], F16, tag="hswin")
                    for tl in range(SB):
                        t = bi * SB + tl
                        xwt = xwt_p.tile([128, KC, B], F16, tag="xwt")
                        if bi == 0:
                            if tl < 8:
                                xw_src = xwblk_d[0][:, :, ts(tl, B)]
                            elif tl < 16:
                                xw_src = xwblk_d[1][:, :, ts(tl - 8, B)]
                            else:
                                xw_src = xwblk_d[2][:, :, ts(tl - 16, B)]
                        elif tl < 8:
                            xw_src = xw_ag_d[0][bi, :, :, ts(tl, B)]
                        elif tl < 16:
                            xw_src = xw_ag_d[1][bi, :, :, ts(tl - 8, B)]
                        else:
                            xw_src = xw_ag_d[2][bi, :, :, ts(tl - 16, B)]
                        nc.sync.dma_start(xwt[:], xw_src)
                        for m in range(KC):
                            pr = ps_rec.tile([128, B], F32, tag="pr")
                            for k in range(KC):
                                if t == 0:
                                    rhs = h0_16[:, k, :]
                                elif tl == 0:
                                    rhs = hw_prev[:, k, ts(SB - 1, B)]
                                else:
                                    rhs = hw[:, k, ts(tl - 1, B)]
                                nc.tensor.matmul(
                                    pr[:],
                                    lhsT=whh16[:, k, ts(m, 128)],
                                    rhs=rhs,
                                    start=(k == 0),
                                    stop=(k == KC - 1),
                                )
                            zp = zpre_p.tile([128, B], F16, tag="zp")
                            nc.vector.tensor_tensor(
                                zp[:], pr[:], xwt[:, m, :], mybir.AluOpType.add
                            )
                            nc.scalar.activation(hw[:, m, ts(tl, B)], zp[:], Tanh)
                            if t == S - 1:
                                nc.scalar.activation(hfin[:, m, :], zp[:], Tanh)
                    # ship this hs block to the bounce buffer
                    nc.sync.dma_start(bin_d[bi][:], hw[:])
                    if bi == NBATCH - 1:
                        nc.sync.dma_start(hT_out[:], hfin[:])
                    hw_prev = hw

                nc.gpsimd.collective_compute(
                    "AllReduce",
                    mybir.AluOpType.add,
                    replica_groups=[list(range(NCORES))],
                    ins=[bin_d[bi][:]],
                    outs=[bout_d[bi][:]],
                )

                with tc.If(pid != 0) as cmp2:
                    hs_rx = hsrx_p.tile([128, KC, SB * B], F16, tag="hsrx")
                    nc.sync.dma_start(hs_rx[:], bout_d[bi][:])
                    for n in range(NT):
                        who16 = who_p.tile([128, KC, 512], F16, tag="who16")
                        nc.sync.dma_start(who16[:], who16_r[:, :, ts(n, 512)])
                        for ml in range(SB * B // 128):
                            pp = ps_prj.tile([128, 512], F32, tag="pp")
                            for k in range(KC):
                                nc.tensor.matmul(
                                    pp[:],
                                    lhsT=hs_rx[:, k, ts(ml, 128)],
                                    rhs=who16[:, k, :],
                                    start=(k == 0),
                                    stop=(k == KC - 1),
                                )
                            ov = evac_p.tile([128, 512], F32, tag="ov")
                            nc.vector.tensor_tensor(
                                ov[:], pp[:], bo_bc[:, ts(n, 512)], mybir.AluOpType.add
                            )
                            mB = bi * (SB * B // 128) + ml
                            nc.sync.dma_start(out_d[ts(mB, 128), ts(n, 512)], ov[:])

    nc.compile()
    return nc


def _get_nc():
    if "nc" not in _cache:
        _install_profile_hook()
        _cache["nc"] = _build()
    return _cache["nc"]


def kernel(x, h, emb, w_ih, w_hh, b_h, w_ho, b_o, trace=False):
    nc = _get_nc()

    x = np.asarray(x)
    # time-major token order: j = s*B + b; core c gathers tokens c*512..c*512+511
    x_tm = x.T.reshape(-1).astype(np.int32)  # [4096]
    hT0 = np.ascontiguousarray(np.asarray(h).T.astype(np.float32))  # [H, B]
    emb = np.ascontiguousarray(np.asarray(emb), dtype=np.float32)
    w_ih = np.ascontiguousarray(np.asarray(w_ih), dtype=np.float32)
    w_hh = np.ascontiguousarray(np.asarray(w_hh), dtype=np.float32)
    b_h = np.ascontiguousarray(np.asarray(b_h), dtype=np.float32)
    w_ho = np.asarray(w_ho, dtype=np.float32)
    b_o = np.asarray(b_o, dtype=np.float32)

    # vocab shards for projection cores 1..7 (core 0 gets zeros, unused)
    who_sh = [np.zeros((H, VSH), np.float32)]
    bo_sh = [np.zeros((VSH,), np.float32)]
    for c in range(NPROJ):
        lo = c * VREAL
        hi = min(lo + VREAL, VOCAB)
        wsl = np.zeros((H, VSH), np.float32)
        wsl[:, : hi - lo] = w_ho[:, lo:hi]
        bsl = np.zeros((VSH,), np.float32)
        bsl[: hi - lo] = b_o[lo:hi]
        who_sh.append(wsl)
        bo_sh.append(bsl)

    in_maps = []
    for c in range(NCORES):
        idx_c = np.ascontiguousarray(
            x_tm[c * 512 : (c + 1) * 512].reshape(4, 128).T
        )  # [128, 4]
        in_maps.append(
            {
                "x_idx": idx_c,
                "hT0": hT0,
                "emb": emb,
                "w_ih": w_ih,
                "w_hh": w_hh,
                "b_h": b_h,
                "w_ho": who_sh[c],
                "b_o": bo_sh[c],
            }
        )

    res = run_bass_kernel_spmd(nc, in_maps, list(range(NCORES)), trace=trace)
    if trace:
        kernel.last_exec_time_ns = res.exec_time_ns

    full = np.empty((BS, VOCAB), np.float32)
    for c in range(NPROJ):
        lo = c * VREAL
        hi = min(lo + VREAL, VOCAB)
        full[:, lo:hi] = res.results[c + 1]["out"][:, : hi - lo]
    outputs = np.ascontiguousarray(
        full.reshape(S, B, VOCAB).transpose(1, 0, 2)
    )  # [B, S, V]
    hT_fin = res.results[0]["hT_fin"]  # [128, KC, B]
    h_final = np.ascontiguousarray(hT_fin.transpose(2, 1, 0).reshape(B, H))
    return outputs, h_final


kernel.last_exec_time_ns = None


# revision 22
# speedup vs baseline: 1.0972x; 1.0972x over previous
"""Trainium2 Bass kernel for a basic RNN LM step:
    xe = emb[x]; xw = xe @ w_ih + b_h
    h_t = tanh(xw_t + h_{t-1} @ w_hh)   (sequential over S=256)
    out = hs @ w_ho + b_o               (dominant GEMM, vocab-sharded)

Split design over 8 NeuronCores (single SPMD program, branches on core id):
  - Phase A (uniform SPMD): each core embeds + transposes + input-projects its
    own 512-token slice (per-core x_idx input), then an AllGather shares the
    xw^T blocks.  Core 0's own block is local, so the recurrence starts
    immediately.
  - Core 0 runs the sequential recurrence (a latency chain of 64 fp16
    weight-reload matmuls per step).  Every 32 steps it broadcasts the new
    hs^T block (1 MB fp16) via AllReduce (other cores contribute zeros).
  - Cores 1..7 tensor-parallel the output projection over vocab columns
    (4572 each, padded to 4608), consuming hs^T blocks as they arrive; w_ho
    is pre-cast to fp16 in DRAM during their initial idle window.
  - All matmuls run in fp16 (1 cycle/row + fast weight load) with fp32 PSUM
    accumulation; measured logit error vs the fp32 reference is ~5e-4
    (absmax-relative).
  - The hidden state is kept transposed (h^T tiles [128, kc, 16]) so the
    recurrence needs no per-step transposes; hs^T blocks double as the
    stationary operand of the projection.
"""

import numpy as np

import concourse.bass as bass
import concourse.mybir as mybir
import concourse.tile as tile
from concourse import bacc
from concourse.bass import ts
from concourse.bass_utils import run_bass_kernel_spmd
from concourse.masks import make_identity

F32 = mybir.dt.float32
F16 = mybir.dt.float16
I32 = mybir.dt.int32

VOCAB = 32000
H = 1024
B = 16
S = 256
NCORES = 8
NPROJ = NCORES - 1  # 7 projection cores
VSH = 4608  # padded per-core vocab shard (ceil(32000/7)=4572 -> 9*512)
VREAL = 4572  # real columns per projection core
KC = H // 128  # 8 chunks of the hidden dim
BS = B * S  # 4096 (time-major: index = s*B + b)
NT = VSH // 512  # 9 projection column tiles of 512
SB = 32  # recurrence steps per broadcast batch / phase-A block
NBATCH = S // SB  # 8 batches == 8 blocks, one per core
Tanh = mybir.ActivationFunctionType.Tanh

_cache = {}


def _install_profile_hook():
    """Make trace=True work under axon if the antenv hook module is absent."""
    import sys
    import types

    try:
        import antenv.axon_hooks  # noqa: F401

        return
    except ImportError:
        pass
    mod = types.ModuleType("antenv.axon_hooks")
    mod._hook = None

    def set_axon_ntff_profile_hook(hook):
        mod._hook = hook

    def get_axon_ntff_profile_hook():
        return mod._hook

    mod.set_axon_ntff_profile_hook = set_axon_ntff_profile_hook
    mod.get_axon_ntff_profile_hook = get_axon_ntff_profile_hook
    sys.modules["antenv.axon_hooks"] = mod
    try:
        from trn_agent_boot.trn_boot import _ntff_profile_via_ctypes

        hook = _ntff_profile_via_ctypes("/opt/axon/libaxon_pjrt.so")
        if hook is not None:
            mod._hook = hook
    except Exception:
        pass


def _build():
    nc = bacc.Bacc("TRN2", target_bir_lowering=False, debug=False, num_devices=NCORES)

    # ---- I/O ----
    # per-core slice of token ids: 512 tokens [128, 4] (core c: tokens c*512..)
    x_idx = nc.declare_dram_parameter("x_idx", [128, 8], I32, isOutput=False)
    hT0_in = nc.declare_dram_parameter("hT0", [H, B], F32, isOutput=False)
    emb_in = nc.declare_dram_parameter("emb", [VOCAB, H], F32, isOutput=False)
    wih_in = nc.declare_dram_parameter("w_ih", [H, H], F32, isOutput=False)
    whh_in = nc.declare_dram_parameter("w_hh", [H, H], F32, isOutput=False)
    bh_in = nc.declare_dram_parameter("b_h", [H], F32, isOutput=False)
    who_in = nc.declare_dram_parameter("w_ho", [H, VSH], F32, isOutput=False)
    bo_in = nc.declare_dram_parameter("b_o", [VSH], F32, isOutput=False)
    out_d = nc.declare_dram_parameter("out", [BS, VSH], F32, isOutput=True)
    hT_out = nc.declare_dram_parameter("hT_fin", [128, KC, B], F32, isOutput=True)

    # internal DRAM
    xwblk_d = [
        nc.dram_tensor(f"xwblk{i}", [128, KC, w], F16)
        for i, w in enumerate((128, 128, 256))
    ]  # local xw^T block, split to keep collective inputs contiguous
    # xw^T AllGathers split by column range so early steps arrive first
    xw_ag_d = [
        nc.dram_tensor(f"xw_ag{i}", [NCORES, 128, KC, w], F16, addr_space="Shared")
        for i, w in enumerate((128, 128, 256))
    ]
    xwblk2_d = nc.dram_tensor("xwblk_nxt", [128, KC, SB * B], F16)  # next block, local only
    who16_d = nc.dram_tensor("who16", [H, VSH], F16)  # fp16 copy of w_ho shard
    who16_r = who16_d.ap().rearrange("(c p) v -> p c v", p=128)
    bin_d = [nc.dram_tensor(f"hsb_in{i}", [128, KC, SB * B], F16) for i in range(NBATCH)]
    bout_d = [
        nc.dram_tensor(f"hsb_out{i}", [128, KC, SB * B], F16, addr_space="Shared")
        for i in range(NBATCH)
    ]

    who_r = who_in.rearrange("(c p) v -> p c v", p=128)
    emb_ap = emb_in[:]

    with tile.TileContext(nc) as tc:
        pid = nc.partition_id()
        with (
            tc.tile_pool(name="consts", bufs=1) as consts,
            tc.tile_pool(name="wstage", bufs=3) as wstage,
            tc.tile_pool(name="gather", bufs=2) as gather_p,
            tc.tile_pool(name="xe16", bufs=2) as xe16_p,
            tc.tile_pool(name="xeT", bufs=1) as xeT_p,
            tc.tile_pool(name="xwt", bufs=6) as xwt_p,
            tc.tile_pool(name="hswin", bufs=2) as hswin_p,
            tc.tile_pool(name="zpre", bufs=8) as zpre_p,
            tc.tile_pool(name="who", bufs=2) as who_p,
            tc.tile_pool(name="hsrx", bufs=2) as hsrx_p,
            tc.tile_pool(name="evac", bufs=2) as evac_p,
            tc.tile_pool(name="ps_t", bufs=1, space="PSUM") as ps_t,
            tc.tile_pool(name="ps_xw", bufs=2, space="PSUM") as ps_xw,
            tc.tile_pool(name="ps_rec", bufs=3, space="PSUM") as ps_rec,
            tc.tile_pool(name="ps_prj", bufs=2, space="PSUM") as ps_prj,
        ):
            # ---- constants (all cores) ----
            ident = consts.tile([128, 128], F16, tag="ident")
            make_identity(nc, ident[:])

            idx_sb = consts.tile([128, 8], I32, tag="idx")
            nc.sync.dma_start(idx_sb[:], x_idx[:])

            bh_sb = consts.tile([128, KC], F32, tag="bh")
            nc.sync.dma_start(bh_sb[:], bh_in.rearrange("(c p) -> p c", p=128))

            # w_ih, w_hh -> fp16 SBUF, chunked [128, KC, H]
            wih16 = consts.tile([128, KC, H], F16, tag="wih16")
            whh16 = consts.tile([128, KC, H], F16, tag="whh16")
            for w16, w_in in ((wih16, wih_in), (whh16, whh_in)):
                wr = w_in.rearrange("(c p) m -> p c m", p=128)
                for c in range(KC):
                    st = wstage.tile([128, H], F32, tag="wst")
                    nc.sync.dma_start(st[:], wr[:, c, :])
                    nc.vector.tensor_copy(w16[:, c, :], st[:])

            # initial hidden state h^T -> fp16 [128, KC, B]
            h0st = wstage.tile([128, KC, B], F32, tag="h0st")
            nc.sync.dma_start(h0st[:], hT0_in.rearrange("(c p) b -> p c b", p=128))
            h0_16 = consts.tile([128, KC, B], F16, tag="h0_16")
            nc.vector.tensor_copy(h0_16[:], h0st[:])

            hfin = consts.tile([128, KC, B], F32, tag="hfin")

            # ---- phase A (uniform): embed + transpose + xw GEMM of own block
            # and of the NEXT core's block (kept local; gives core 0 block 1
            # before the AllGathers complete)
            for half in range(2):
                xeT = xeT_p.tile([128, KC, SB * B], F16, tag="xeT")
                for sub in range(4):
                    ge = gather_p.tile([128, H], F32, tag="ge")
                    nc.gpsimd.indirect_dma_start(
                        out=ge[:],
                        out_offset=None,
                        in_=emb_ap,
                        in_offset=bass.IndirectOffsetOnAxis(
                            ap=idx_sb[:, half * 4 + sub : half * 4 + sub + 1], axis=0
                        ),
                    )
                    xe16 = xe16_p.tile([128, H], F16, tag="xe16")
                    nc.vector.tensor_copy(xe16[:], ge[:])
                    for k in range(KC):
                        pt = ps_t.tile([128, 128], F16, tag="pt")
                        nc.tensor.transpose(pt[:], xe16[:, ts(k, 128)], ident[:])
                        nc.vector.tensor_copy(xeT[:, k, ts(sub, 128)], pt[:])
                for m in range(KC):
                    pxw = ps_xw.tile([128, SB * B], F32, tag="pxw")
                    for k in range(KC):
                        nc.tensor.matmul(
                            pxw[:],
                            lhsT=wih16[:, k, ts(m, 128)],
                            rhs=xeT[:, k, :],
                            start=(k == 0),
                            stop=(k == KC - 1),
                        )
                    xv = evac_p.tile([128, SB * B], F16, tag="xv")
                    nc.scalar.add(xv[:], pxw[:], bh_sb[:, m : m + 1])
                    if half == 0:
                        for agi, (lo, w) in enumerate(((0, 128), (128, 128), (256, 256))):
                            nc.sync.dma_start(xwblk_d[agi][:, m, :], xv[:, lo : lo + w])
                    else:
                        nc.sync.dma_start(xwblk2_d[:, m, :], xv[:])

            # share the xw blocks: xw_ag*[c] = core c's block (column-split)
            for agi in range(3):
                nc.gpsimd.collective_compute(
                    "AllGather",
                    mybir.AluOpType.bypass,
                    replica_groups=[list(range(NCORES))],
                    ins=[xwblk_d[agi][:]],
                    outs=[xw_ag_d[agi][:]],
                )

            # ---- non-root setup: zero collective inputs, b_o bcast, w_ho cast
            with tc.If(pid != 0) as cmp0:
                zt = consts.tile([128, KC, SB * B], F16, tag="zt")
                nc.vector.memset(zt[:], 0.0)
                for i in range(NBATCH):
                    nc.sync.dma_start(bin_d[i][:], zt[:])
                bo_bc = consts.tile([128, VSH], F32, tag="bo_bc")
                for p in range(128):
                    nc.sync.dma_start(bo_bc[p : p + 1, :], bo_in[:].unsqueeze(0))
                for c in range(KC):
                    for q in range(4):
                        st = wstage.tile([128, VSH // 4], F32, tag="whost")
                        nc.sync.dma_start(st[:], who_r[:, c, ts(q, VSH // 4)])
                        s16 = wstage.tile([128, VSH // 4], F16, tag="whost16")
                        nc.vector.tensor_copy(s16[:], st[:])
                        nc.sync.dma_start(who16_r[:, c, ts(q, VSH // 4)], s16[:])

            # ================= recurrence + broadcast + projection ============
            hw_prev = None
            for bi in range(NBATCH):
                with tc.If(pid == 0) as cmp:
                    hw = hswin_p.tile([128, KC, SB * B], F16, tag="hswin")
                    for tl in range(SB):
                        t = bi * SB + tl
                        xwt = xwt_p.tile([128, KC, B], F16, tag="xwt")
                        if bi == 0:
                            if tl < 8:
                                xw_src = xwblk_d[0][:, :, ts(tl, B)]
                            elif tl < 16:
                                xw_src = xwblk_d[1][:, :, ts(tl - 8, B)]
                            else:
                                xw_src = xwblk_d[2][:, :, ts(tl - 16, B)]
                        elif bi == 1:
                            xw_src = xwblk2_d[:, :, ts(tl, B)]
                        elif tl < 8:
                            xw_src = xw_ag_d[0][bi, :, :, ts(tl, B)]
                        elif tl < 16:
                            xw_src = xw_ag_d[1][bi, :, :, ts(tl - 8, B)]
                        else:
                            xw_src = xw_ag_d[2][bi, :, :, ts(tl - 16, B)]
                        nc.sync.dma_start(xwt[:], xw_src)
                        for m in range(KC):
                            pr = ps_rec.tile([128, B], F32, tag="pr")
                            for k in range(KC):
                                if t == 0:
                                    rhs = h0_16[:, k, :]
                                elif tl == 0:
                                    rhs = hw_prev[:, k, ts(SB - 1, B)]
                                else:
                                    rhs = hw[:, k, ts(tl - 1, B)]
                                nc.tensor.matmul(
                                    pr[:],
                                    lhsT=whh16[:, k, ts(m, 128)],
                                    rhs=rhs,
                                    start=(k == 0),
                                    stop=(k == KC - 1),
                                )
                            zp = zpre_p.tile([128, B], F16, tag="zp")
                            nc.vector.tensor_tensor(
                                zp[:], pr[:], xwt[:, m, :], mybir.AluOpType.add
                            )
                            nc.scalar.activation(hw[:, m, ts(tl, B)], zp[:], Tanh)
                            if t == S - 1:
                                nc.scalar.activation(hfin[:, m, :], zp[:], Tanh)
                    # ship this hs block to the bounce buffer
                    nc.sync.dma_start(bin_d[bi][:], hw[:])
                    if bi == NBATCH - 1:
                        nc.sync.dma_start(hT_out[:], hfin[:])
                    hw_prev = hw

                nc.gpsimd.collective_compute(
                    "AllReduce",
                    mybir.AluOpType.add,
                    replica_groups=[list(range(NCORES))],
                    ins=[bin_d[bi][:]],
                    outs=[bout_d[bi][:]],
                )

                with tc.If(pid != 0) as cmp2:
                    hs_rx = hsrx_p.tile([128, KC, SB * B], F16, tag="hsrx")
                    nc.sync.dma_start(hs_rx[:], bout_d[bi][:])
                    for n in range(NT):
                        who16 = who_p.tile([128, KC, 512], F16, tag="who16")
                        nc.sync.dma_start(who16[:], who16_r[:, :, ts(n, 512)])
                        for ml in range(SB * B // 128):
                            pp = ps_prj.tile([128, 512], F32, tag="pp")
                            for k in range(KC):
                                nc.tensor.matmul(
                                    pp[:],
                                    lhsT=hs_rx[:, k, ts(ml, 128)],
                                    rhs=who16[:, k, :],
                                    start=(k == 0),
                                    stop=(k == KC - 1),
                                )
                            ov = evac_p.tile([128, 512], F32, tag="ov")
                            nc.vector.tensor_tensor(
                                ov[:], pp[:], bo_bc[:, ts(n, 512)], mybir.AluOpType.add
                            )
                            mB = bi * (SB * B // 128) + ml
                            nc.sync.dma_start(out_d[ts(mB, 128), ts(n, 512)], ov[:])

    nc.compile()
    return nc


def _get_nc():
    if "nc" not in _cache:
        _install_profile_hook()
        _cache["nc"] = _build()
    return _cache["nc"]


def kernel(x, h, emb, w_ih, w_hh, b_h, w_ho, b_o, trace=False):
    nc = _get_nc()

    x = np.asarray(x)
    # time-major token order: j = s*B + b; core c gathers tokens c*512..c*512+511
    x_tm = x.T.reshape(-1).astype(np.int32)  # [4096]
    hT0 = np.ascontiguousarray(np.asarray(h).T.astype(np.float32))  # [H, B]
    emb = np.ascontiguousarray(np.asarray(emb), dtype=np.float32)
    w_ih = np.ascontiguousarray(np.asarray(w_ih), dtype=np.float32)
    w_hh = np.ascontiguousarray(np.asarray(w_hh), dtype=np.float32)
    b_h = np.ascontiguousarray(np.asarray(b_h), dtype=np.float32)
    w_ho = np.asarray(w_ho, dtype=np.float32)
    b_o = np.asarray(b_o, dtype=np.float32)

    # vocab shards for projection cores 1..7 (core 0 gets zeros, unused)
    who_sh = [np.zeros((H, VSH), np.float32)]
    bo_sh = [np.zeros((VSH,), np.float32)]
    for c in range(NPROJ):
        lo = c * VREAL
        hi = min(lo + VREAL, VOCAB)
        wsl = np.zeros((H, VSH), np.float32)
        wsl[:, : hi - lo] = w_ho[:, lo:hi]
        bsl = np.zeros((VSH,), np.float32)
        bsl[: hi - lo] = b_o[lo:hi]
        who_sh.append(wsl)
        bo_sh.append(bsl)

    in_maps = []
    for c in range(NCORES):
        c2 = (c + 1) % NCORES
        idx_c = np.ascontiguousarray(
            np.concatenate(
                [
                    x_tm[c * 512 : (c + 1) * 512].reshape(4, 128),
                    x_tm[c2 * 512 : (c2 + 1) * 512].reshape(4, 128),
                ]
            ).T
        )  # [128, 8]
        in_maps.append(
            {
                "x_idx": idx_c,
                "hT0": hT0,
                "emb": emb,
                "w_ih": w_ih,
                "w_hh": w_hh,
                "b_h": b_h,
                "w_ho": who_sh[c],
                "b_o": bo_sh[c],
            }
        )

    res = run_bass_kernel_spmd(nc, in_maps, list(range(NCORES)), trace=trace)
    if trace:
        kernel.last_exec_time_ns = res.exec_time_ns

    full = np.empty((BS, VOCAB), np.float32)
    for c in range(NPROJ):
        lo = c * VREAL
        hi = min(lo + VREAL, VOCAB)
        full[:, lo:hi] = res.results[c + 1]["out"][:, : hi - lo]
    outputs = np.ascontiguousarray(
        full.reshape(S, B, VOCAB).transpose(1, 0, 2)
    )  # [B, S, V]
    hT_fin = res.results[0]["hT_fin"]  # [128, KC, B]
    h_final = np.ascontiguousarray(hT_fin.transpose(2, 1, 0).reshape(B, H))
    return outputs, h_final


kernel.last_exec_time_ns = None
